# revision 10
# baseline (speedup 1.0000x reference)
"""Trainium2 Bass kernel for group-dequantized linear (AxCoreDSEWLinear).

Computes y = x @ (weight * group_scales).T + bias on 8 NeuronCores,
column-parallel over out_features (1024 per core).

Math per core (o-shard of 1024 columns):
  - weight is shipped host-transposed as WT [I=8192, O=1024] so the
    contraction dim i lands on SBUF partitions with no on-device transpose.
  - For each in-channel group g (128 channels = 1 partition tile) a single
    matmul produces the partial p_g[b, o] = sum_{i in g} x[b,i] W[o,i].
    Partials for 4 groups are stacked into one PSUM bank at partition bases
    {0, 32, 64, 96} (batch dim padded 16 -> 32 with zero columns in lhsT).
  - DVE multiplies the stacked partials by the replicated per-(o, g) scales.
  - A second "selection" matmul S.T @ scaled_p sums the 4 group blocks and
    accumulates all 16 stacks in PSUM: y[b,o] = sum_g scale[o,g] p_g[b,o].
  - Bias is added during the PSUM->SBUF move, then DMAed out.
"""

import os
import numpy as np

B = 16
I = 8192
O = 8192
NCORES = 8
OS = O // NCORES          # 1024 out features per core
G = 128                   # in-channel group size
NG = I // G               # 64 groups
QPS = 4                   # groups stacked per PSUM tile
NSTACK = NG // QPS        # 16 stacks
CH = 512                  # o-chunk (fp32 moving-operand max)
NCH = OS // CH            # 2 chunks

_prog_cache: dict = {}

# results of the most recent traced run (for test harness use)
last_exec_time_ns = None
last_profile = None


def _build_program(use_f32r: bool):
    import concourse.bacc as bacc
    import concourse.mybir as mybir
    import concourse.tile as tile

    f32 = mybir.dt.float32

    def mmdt(ap):
        return ap.bitcast(mybir.dt.float32r) if use_f32r else ap

    # Bacc (not plain Bass): its finalize() runs generate_event_semaphores,
    # which splits multi-wait instructions — this walrus build caps every
    # instruction at one sync wait.
    nc = bacc.Bacc()
    wt = nc.dram_tensor("wt", [I, OS], f32, kind="ExternalInput")
    xt = nc.dram_tensor("xt", [128, NG * 32], f32, kind="ExternalInput")
    srep = nc.dram_tensor("srep", [NSTACK, 128, OS], f32, kind="ExternalInput")
    s_sel = nc.dram_tensor("s_sel", [128, B], f32, kind="ExternalInput")
    biasr = nc.dram_tensor("biasr", [B, OS], f32, kind="ExternalInput")
    y = nc.dram_tensor("y", [B, OS], f32, kind="ExternalOutput")

    with tile.TileContext(nc) as tc:
        with (
            tc.tile_pool(name="const", bufs=1) as const_pool,
            tc.tile_pool(name="wtp", bufs=3) as wt_pool,
            tc.tile_pool(name="spp", bufs=3) as sp_pool,
            tc.tile_pool(name="outp", bufs=2) as out_pool,
            tc.tile_pool(name="pp", bufs=3, space="PSUM") as psum_p,
            tc.tile_pool(name="py", bufs=2, space="PSUM") as psum_y,
        ):
            xt_sb = const_pool.tile([128, NG * 32], f32, tag="xt")
            nc.sync.dma_start(xt_sb[:], xt[:])
            s_sb = const_pool.tile([128, B], f32, tag="s_sel")
            nc.sync.dma_start(s_sb[:], s_sel[:])
            bias_sb = const_pool.tile([B, OS], f32, tag="bias")
            nc.sync.dma_start(bias_sb[:], biasr[:])
            srep_sb = const_pool.tile([128, NSTACK * OS], f32, tag="srep")
            for s in range(NSTACK):
                nc.sync.dma_start(srep_sb[:, s * OS : (s + 1) * OS], srep[s])

            y_ps = [
                psum_y.tile([B, CH], f32, tag=f"y{ch}", name=f"y_ps{ch}")
                for ch in range(NCH)
            ]

            for s in range(NSTACK):
                wt_t = wt_pool.tile([128, QPS * OS], f32, tag="wt")
                for q in range(QPS):
                    g = QPS * s + q
                    nc.sync.dma_start(
                        wt_t[:, q * OS : (q + 1) * OS],
                        wt[g * G : (g + 1) * G, :],
                    )
                for ch in range(NCH):
                    p_ps = psum_p.tile([128, CH], f32, tag="p")
                    for q in range(QPS):
                        g = QPS * s + q
                        nc.tensor.matmul(
                            p_ps[32 * q : 32 * (q + 1), :],
                            mmdt(xt_sb[:, g * 32 : (g + 1) * 32]),
                            mmdt(wt_t[:, q * OS + ch * CH : q * OS + ch * CH + CH]),
                            start=True,
                            stop=True,
                            tile_position=(0, 32 * q),
                        )
                    sp_t = sp_pool.tile([128, CH], f32, tag="sp")
                    nc.vector.tensor_mul(
                        sp_t[:],
                        p_ps[:],
                        srep_sb[:, s * OS + ch * CH : s * OS + ch * CH + CH],
                    )
                    nc.tensor.matmul(
                        y_ps[ch][:],
                        mmdt(s_sb[:]),
                        mmdt(sp_t[:]),
                        start=(s == 0),
                        stop=(s == NSTACK - 1),
                    )

            for ch in range(NCH):
                y_sb = out_pool.tile([B, CH], f32, tag="y_sb")
                nc.vector.tensor_add(
                    y_sb[:], y_ps[ch][:], bias_sb[:, ch * CH : (ch + 1) * CH]
                )
                nc.sync.dma_start(y[:, ch * CH : (ch + 1) * CH], y_sb[:])

    nc.finalize()
    return nc


def _ensure_ntff_hook():
    """Provide antenv.axon_hooks if the image lacks it (trace-only path).

    run_bass_kernel_spmd(trace=True) under axon imports
    antenv.axon_hooks.get_axon_ntff_profile_hook; this image's antenv has no
    axon_hooks module, so register an equivalent ctypes-based hook over the
    axon PJRT .so (same ABI the boot shim uses).
    """
    import sys
    import types
    import ctypes
    import contextlib

    try:
        from antenv.axon_hooks import get_axon_ntff_profile_hook  # noqa: F401
        return
    except ImportError:
        pass

    so_path = "/opt/axon/libaxon_pjrt.so"
    hook = None
    if os.path.exists(so_path):
        lib = ctypes.CDLL(so_path)
        if hasattr(lib, "axon_start_nrt_profile"):
            lib.axon_start_nrt_profile.argtypes = [
                ctypes.POINTER(ctypes.c_int64),
                ctypes.c_size_t,
            ]
            lib.axon_start_nrt_profile.restype = ctypes.c_int64
            lib.axon_stop_nrt_profile.argtypes = [ctypes.c_char_p]
            lib.axon_stop_nrt_profile.restype = ctypes.c_int64

            @contextlib.contextmanager
            def _hook(output_dir, device_ids):
                import jax

                jax.devices()
                if device_ids:
                    ids = (ctypes.c_int64 * len(device_ids))(*device_ids)
                    rc = lib.axon_start_nrt_profile(ids, len(device_ids))
                else:
                    rc = lib.axon_start_nrt_profile(None, 0)
                if rc != 0:
                    raise RuntimeError(f"axon_start_nrt_profile rc={rc}")
                try:
                    yield
                finally:
                    n = lib.axon_stop_nrt_profile(str(output_dir).encode())
                    print(f"profile: {n} file(s) written to {output_dir}")

            hook = _hook

    mod = types.ModuleType("antenv.axon_hooks")
    mod._hook = hook

    def set_axon_ntff_profile_hook(h):
        mod._hook = h

    def get_axon_ntff_profile_hook():
        return mod._hook

    mod.set_axon_ntff_profile_hook = set_axon_ntff_profile_hook
    mod.get_axon_ntff_profile_hook = get_axon_ntff_profile_hook
    sys.modules["antenv.axon_hooks"] = mod


def _host_prep(x, weight, scale_buf, bias):
    """Build per-core input maps (numpy layout prep only)."""
    x = np.ascontiguousarray(x, dtype=np.float32)
    weight = np.ascontiguousarray(weight, dtype=np.float32)
    scale_buf = np.ascontiguousarray(scale_buf, dtype=np.float32)
    bias = np.ascontiguousarray(bias, dtype=np.float32)

    # xt tiles: xt[p, g*32 + j] = x[j, g*128 + p] for j < 16, else 0
    xr = x.reshape(B, NG, G).transpose(2, 1, 0)          # [128, 64, 16]
    xt = np.zeros((G, NG, 32), dtype=np.float32)
    xt[:, :, :B] = xr
    xt = np.ascontiguousarray(xt.reshape(G, NG * 32))

    s_sel = np.zeros((128, B), dtype=np.float32)
    for q in range(QPS):
        s_sel[32 * q + np.arange(B), np.arange(B)] = 1.0

    in_maps = []
    for c in range(NCORES):
        sl = slice(c * OS, (c + 1) * OS)
        wt_c = np.ascontiguousarray(weight[sl, :].T)     # [I, OS]
        scale_t = scale_buf[sl, :].T                     # [NG, OS]
        srep_c = np.ascontiguousarray(
            np.broadcast_to(
                scale_t.reshape(NSTACK, QPS, 1, OS), (NSTACK, QPS, 32, OS)
            ).reshape(NSTACK, 128, OS)
        )
        bias_c = np.ascontiguousarray(
            np.broadcast_to(bias.reshape(O)[sl][None, :], (B, OS))
        )
        in_maps.append(
            {"wt": wt_c, "xt": xt, "srep": srep_c, "s_sel": s_sel, "biasr": bias_c}
        )
    return in_maps


def kernel(x, weight, scale_buf, bias, types):
    """Full-input entry point: returns y = x @ (weight*scales).T + bias."""
    global last_exec_time_ns, last_profile
    from concourse.bass_utils import run_bass_kernel_spmd

    use_f32r = os.environ.get("KB_F32R", "1") == "1"
    trace = os.environ.get("KB_TRACE", "0") == "1"
    if trace:
        _ensure_ntff_hook()

    key = ("prog", use_f32r)
    if key not in _prog_cache:
        _prog_cache[key] = _build_program(use_f32r)
    nc = _prog_cache[key]

    in_maps = _host_prep(x, weight, scale_buf, bias)
    res = run_bass_kernel_spmd(nc, in_maps, list(range(NCORES)), trace=trace)
    last_exec_time_ns = res.exec_time_ns
    last_profile = res.profile_json

    out = np.concatenate(
        [res.results[c]["y"] for c in range(NCORES)], axis=1
    ).astype(np.float32, copy=False)
    return out
